# revision 3
# baseline (speedup 1.0000x reference)
"""Trainium2 Bass kernel for a differentiable GRU decoder.

Per step t (max_len=32 steps), batch N=4096, E=512, V=1024:
    emb    = probs_{t-1} @ W_d2e.T            # [N, E]
    h      = GRUCell(emb, h)                  # [N, E]
    logits = h @ W_e2d.T + b_e2d              # [N, V]
    probs  = softmax(logits)                  # [N, V]  -> output[t]

Sharding: data-parallel over N across 8 cores (512 rows each), weights
replicated, the 32-step scan stays local per core — no collectives.

On-chip layout is feature-major ([features on partitions, batch on free])
so every matmul chains without transposes; the per-core output is written
feature-major as [T, V, 512] and un-transposed on the host during the
gather.  All matmuls run as float32r (fp32 data rounded to the PE's fast
fp32 format: ~1.6e-4 relative, 1 cycle/row at free-dim 512, 4x faster
than strict fp32).  Softmax row sums (a reduction over partitions) are
computed with a ones-vector matmul; 1/sum is broadcast back across
partitions with a rank-1 ones-outer-product matmul.  The normalization
of the NEXT step's emb input is folded into the PSUM->SBUF drain of the
emb matmul (exp(logits) @ W' scaled by 1/sum == softmax @ W'), so the
unnormalized exp tiles feed the matmul and the fp32 output tiles get a
full-precision normalize.
"""

import sys
import types

import numpy as np

import concourse.bacc as bacc
import concourse.mybir as mybir
import concourse.tile as tile

F32 = mybir.dt.float32
F32R = mybir.dt.float32r
AF = mybir.ActivationFunctionType

N_CORES = 8


def _install_ntff_hook():
    """Register the axon NTFF profiling hook if the image's antenv lacks it."""
    try:
        import antenv.axon_hooks  # noqa: F401
        return
    except ImportError:
        pass
    try:
        from trn_agent_boot.trn_boot import _ntff_profile_via_ctypes

        hook = _ntff_profile_via_ctypes("/opt/axon/libaxon_pjrt.so")
    except Exception:
        hook = None
    mod = types.ModuleType("antenv.axon_hooks")
    mod.get_axon_ntff_profile_hook = lambda: hook
    mod.set_axon_ntff_profile_hook = lambda h: None
    sys.modules["antenv.axon_hooks"] = mod


_install_ntff_hook()


def _build(T, B, E, V):
    """Build the per-core Bacc module. B = per-core batch (free dim)."""
    KE = E // 128  # E-tiles (4)
    KV = V // 128  # V-tiles (8)

    nc = bacc.Bacc(None, target_bir_lowering=False)

    xT = nc.dram_tensor("xT", [E, B], F32, kind="ExternalInput")
    wd2eT = nc.dram_tensor("wd2eT", [V, E], F32, kind="ExternalInput")
    wihT = nc.dram_tensor("wihT", [E, 3 * E], F32, kind="ExternalInput")
    whhT = nc.dram_tensor("whhT", [E, 3 * E], F32, kind="ExternalInput")
    we2dT = nc.dram_tensor("we2dT", [E, V], F32, kind="ExternalInput")
    brz = nc.dram_tensor("brz", [128, 2 * KE], F32, kind="ExternalInput")
    bihn = nc.dram_tensor("bihn", [128, KE], F32, kind="ExternalInput")
    bhhn = nc.dram_tensor("bhhn", [128, KE], F32, kind="ExternalInput")
    be2d = nc.dram_tensor("be2d", [128, KV], F32, kind="ExternalInput")
    out = nc.dram_tensor("out", [T, V, B], F32, kind="ExternalOutput")

    with tile.TileContext(nc) as tc:
        with (
            tc.tile_pool(name="w", bufs=1) as wp,
            tc.tile_pool(name="sb", bufs=1) as sb,
            tc.tile_pool(name="ps", bufs=1, space="PSUM") as pp,
        ):
            # ---- persistent weights (rounded to f32r via a copy) ----
            def load_f32r(name, dram_ap, rows, cols):
                st = sb.tile([128, cols], F32, name="stage", tag="stage", bufs=2)
                nc.sync.dma_start(st[:, :], dram_ap)
                wt = wp.tile([128, cols], F32R, name=name, tag=name)
                nc.vector.tensor_copy(wt[:], st[:])
                return wt

            w_d2e = [
                load_f32r(f"w_d2e{k}", wd2eT[k * 128 : (k + 1) * 128, :], 128, E)
                for k in range(KV)
            ]
            w_ih = [
                load_f32r(f"w_ih{k}", wihT[k * 128 : (k + 1) * 128, :], 128, 3 * E)
                for k in range(KE)
            ]
            w_hh = [
                load_f32r(f"w_hh{k}", whhT[k * 128 : (k + 1) * 128, :], 128, 3 * E)
                for k in range(KE)
            ]
            w_e2d = [
                load_f32r(f"w_e2d{k}", we2dT[k * 128 : (k + 1) * 128, :], 128, V)
                for k in range(KE)
            ]

            b_rz = wp.tile([128, 2 * KE], F32, name="b_rz", tag="b_rz")
            nc.sync.dma_start(b_rz[:], brz[:])
            b_ihn = wp.tile([128, KE], F32, name="b_ihn", tag="b_ihn")
            nc.sync.dma_start(b_ihn[:], bihn[:])
            b_hhn = wp.tile([128, KE], F32, name="b_hhn", tag="b_hhn")
            nc.sync.dma_start(b_hhn[:], bhhn[:])
            b_e2d = wp.tile([128, KV], F32, name="b_e2d", tag="b_e2d")
            nc.sync.dma_start(b_e2d[:], be2d[:])

            ones_f32 = wp.tile([128, 1], F32, name="ones_f32", tag="ones_f32")
            nc.gpsimd.memset(ones_f32[:], 1.0)
            ones_col = wp.tile([128, 1], F32R, name="ones_col", tag="ones_col")
            nc.vector.tensor_copy(ones_col[:], ones_f32[:])
            ones_row_f32 = wp.tile([1, 128], F32, name="ones_row_f32", tag="ones_row_f32")
            nc.gpsimd.memset(ones_row_f32[:], 1.0)
            ones_row = wp.tile([1, 128], F32R, name="ones_row", tag="ones_row")
            nc.vector.tensor_copy(ones_row[:], ones_row_f32[:])

            # ---- initial state h = x (feature-major, rounded to f32r) ----
            hT = []
            for m in range(KE):
                st = sb.tile([128, B], F32, name="stage", tag="stage", bufs=2)
                nc.sync.dma_start(st[:, :B], xT[m * 128 : (m + 1) * 128, :])
                ht = sb.tile([128, B], F32R, name="h", tag="h", bufs=8)
                nc.vector.tensor_copy(ht[:], st[:, :B])
                hT.append(ht)

            eT = None  # unnormalized exp(logits) of previous step (f32r)
            rbc = None  # 1/rowsum broadcast [128, B] of previous step

            for t in range(T):
                # ---- emb = softmax_{t-1} @ W_d2e.T  (feature-major [E, B]);
                # normalization folded into the PSUM drain ----
                embT = None
                if t > 0:
                    embT = []
                    for m in range(KE):
                        ps = pp.tile([128, B], F32, name="ps_mm", tag="mm", bufs=6)
                        for k in range(KV):
                            nc.tensor.matmul(
                                ps[:],
                                w_d2e[k][:, m * 128 : (m + 1) * 128],
                                eT[k][:],
                                start=(k == 0),
                                stop=(k == KV - 1),
                            )
                        ev = sb.tile([128, B], F32R, name="embT", tag="embT", bufs=8)
                        nc.vector.tensor_mul(ev[:], ps[:], rbc[:])
                        embT.append(ev)

                # ---- gates r, z: sigmoid(gx + gh + b_ih + b_hh) ----
                rz = []
                for g in range(2):
                    gt_list = []
                    for m in range(KE):
                        col = g * E + m * 128
                        ps = pp.tile([128, B], F32, name="ps_mm", tag="mm", bufs=6)
                        first = True
                        if t > 0:
                            for k in range(KE):
                                nc.tensor.matmul(
                                    ps[:],
                                    w_ih[k][:, col : col + 128],
                                    embT[k][:],
                                    start=first,
                                    stop=False,
                                )
                                first = False
                        for k in range(KE):
                            nc.tensor.matmul(
                                ps[:],
                                w_hh[k][:, col : col + 128],
                                hT[k][:],
                                start=first,
                                stop=(k == KE - 1),
                            )
                            first = False
                        gt = sb.tile(
                            [128, B], F32, name=f"gate{g}", tag=f"gate{g}", bufs=4
                        )
                        j = g * KE + m
                        nc.scalar.activation(
                            gt[:], ps[:], AF.Sigmoid, bias=b_rz[:, j : j + 1]
                        )
                        gt_list.append(gt)
                    rz.append(gt_list)
                r_g, z_g = rz

                # ---- n gate: tanh(xn + b_ihn + r * (hn + b_hhn)) ----
                xnb = None
                if t > 0:
                    xnb = []
                    for m in range(KE):
                        col = 2 * E + m * 128
                        ps = pp.tile([128, B], F32, name="ps_mm", tag="mm", bufs=6)
                        for k in range(KE):
                            nc.tensor.matmul(
                                ps[:],
                                w_ih[k][:, col : col + 128],
                                embT[k][:],
                                start=(k == 0),
                                stop=(k == KE - 1),
                            )
                        xv = sb.tile([128, B], F32, name="xnb", tag="xnb", bufs=4)
                        nc.scalar.activation(
                            xv[:], ps[:], AF.Identity, bias=b_ihn[:, m : m + 1]
                        )
                        xnb.append(xv)

                n_g = []
                for m in range(KE):
                    col = 2 * E + m * 128
                    ps = pp.tile([128, B], F32, name="ps_mm", tag="mm", bufs=6)
                    for k in range(KE):
                        nc.tensor.matmul(
                            ps[:],
                            w_hh[k][:, col : col + 128],
                            hT[k][:],
                            start=(k == 0),
                            stop=(k == KE - 1),
                        )
                    hv = sb.tile([128, B], F32, name="hnb", tag="hnb", bufs=4)
                    nc.scalar.activation(
                        hv[:], ps[:], AF.Identity, bias=b_hhn[:, m : m + 1]
                    )
                    n_g.append(hv)

                for m in range(KE):
                    # t1 = r * (hn + b_hhn), in place into n_g
                    nc.vector.tensor_mul(n_g[m][:], r_g[m][:], n_g[m][:])
                if t > 0:
                    for m in range(KE):
                        nc.vector.tensor_add(n_g[m][:], n_g[m][:], xnb[m][:])
                    for m in range(KE):
                        nc.scalar.activation(n_g[m][:], n_g[m][:], AF.Tanh)
                else:
                    # xn == 0 at t=0; fold b_ihn into the tanh bias
                    for m in range(KE):
                        nc.scalar.activation(
                            n_g[m][:], n_g[m][:], AF.Tanh, bias=b_ihn[:, m : m + 1]
                        )

                # ---- h' = n + z * (h - n) ----
                hN = []
                for m in range(KE):
                    nc.vector.tensor_sub(
                        r_g[m][:], hT[m][:].bitcast(F32), n_g[m][:]
                    )  # d = h - n (reuse r tile)
                    nc.vector.tensor_mul(z_g[m][:], z_g[m][:], r_g[m][:])  # z*d
                    ht = sb.tile([128, B], F32R, name="h", tag="h", bufs=8)
                    nc.vector.tensor_add(ht[:], n_g[m][:], z_g[m][:])
                    hN.append(ht)
                hT = hN

                # ---- logits = h' @ W_e2d.T + b_e2d; eT = exp(logits) ----
                eT = []
                for j in range(KV):
                    ps = pp.tile([128, B], F32, name="ps_mm", tag="mm", bufs=6)
                    for k in range(KE):
                        nc.tensor.matmul(
                            ps[:],
                            w_e2d[k][:, j * 128 : (j + 1) * 128],
                            hT[k][:],
                            start=(k == 0),
                            stop=(k == KE - 1),
                        )
                    ev = sb.tile([128, B], F32R, name="eT", tag="eT", bufs=12)
                    nc.scalar.activation(
                        ev[:], ps[:], AF.Exp, bias=b_e2d[:, j : j + 1]
                    )
                    eT.append(ev)

                # ---- softmax row sums over V (partition reduction) ----
                ps_s = pp.tile([1, B], F32, name="ps_s", tag="srow", bufs=1)
                for j in range(KV):
                    nc.tensor.matmul(
                        ps_s[:],
                        ones_col[:],
                        eT[j][:],
                        start=(j == 0),
                        stop=(j == KV - 1),
                    )
                rinv = sb.tile([1, B], F32R, name="rinv", tag="rinv", bufs=2)
                with nc.allow_low_precision(reason="f32r rounding for PE broadcast"):
                    nc.vector.reciprocal(rinv[:], ps_s[:])
                # broadcast 1/sum across partitions: ones[128,1] x rinv[1,B]
                ps_b = pp.tile([128, B], F32, name="ps_b", tag="bc", bufs=1)
                nc.tensor.matmul(ps_b[:], ones_row[:], rinv[:], start=True, stop=True)
                rbc = sb.tile([128, B], F32, name="rbc", tag="rbc", bufs=2)
                nc.vector.tensor_copy(rbc[:], ps_b[:])

                # ---- probs = exp(logits) / rowsum -> output[t] (fp32) ----
                for j in range(KV):
                    po = sb.tile([128, B], F32, name="pout", tag="pout", bufs=6)
                    nc.vector.tensor_mul(po[:], eT[j][:].bitcast(F32), rbc[:])
                    nc.sync.dma_start(out[t, j * 128 : (j + 1) * 128, :], po[:])

    nc.compile()
    return nc


def _prep_inputs(x, W_d2e, W_ih, W_hh, b_ih, b_hh, W_e2d, b_e2d):
    E = x.shape[1]
    V = np.asarray(W_e2d).shape[0]
    KE = E // 128
    KV = V // 128

    def c(a):
        return np.ascontiguousarray(np.asarray(a), dtype=np.float32)

    shared = {
        "wd2eT": c(np.asarray(W_d2e).T),  # [V, E]
        "wihT": c(np.asarray(W_ih).T),  # [E, 3E]
        "whhT": c(np.asarray(W_hh).T),
        "we2dT": c(np.asarray(W_e2d).T),  # [E, V]
        "brz": c((np.asarray(b_ih) + np.asarray(b_hh))[: 2 * E].reshape(2 * KE, 128).T),
        "bihn": c(np.asarray(b_ih)[2 * E :].reshape(KE, 128).T),
        "bhhn": c(np.asarray(b_hh)[2 * E :].reshape(KE, 128).T),
        "be2d": c(np.asarray(b_e2d).reshape(KV, 128).T),
    }
    N = x.shape[0]
    B = N // N_CORES
    in_maps = []
    for core in range(N_CORES):
        m = dict(shared)
        m["xT"] = c(np.asarray(x)[core * B : (core + 1) * B, :].T)  # [E, B]
        in_maps.append(m)
    return in_maps, B


def _run(inputs, trace=False):
    from concourse.bass_utils import run_bass_kernel_spmd

    x = np.asarray(inputs["x"], dtype=np.float32)
    T = int(inputs["max_len"])
    N, E = x.shape
    V = np.asarray(inputs["W_e2d"]).shape[0]
    assert N % N_CORES == 0 and E % 128 == 0 and V % 128 == 0

    in_maps, B = _prep_inputs(
        x,
        inputs["W_d2e"],
        inputs["W_ih"],
        inputs["W_hh"],
        inputs["b_ih"],
        inputs["b_hh"],
        inputs["W_e2d"],
        inputs["b_e2d"],
    )
    nc = _build(T, B, E, V)
    res = run_bass_kernel_spmd(
        nc, in_maps, core_ids=list(range(N_CORES)), trace=trace
    )

    full = np.empty((T, N, V), dtype=np.float32)
    for core in range(N_CORES):
        o = res.results[core]["out"]  # [T, V, B]
        full[:, core * B : (core + 1) * B, :] = np.transpose(o, (0, 2, 1))
    return full, res


def kernel(**inputs):
    full, _ = _run(inputs, trace=False)
    return full


def run_traced(**inputs):
    return _run(inputs, trace=True)


# revision 8
# speedup vs baseline: 1.1941x; 1.1941x over previous
"""Trainium2 Bass kernel for a differentiable GRU decoder.

Per step t (max_len=32 steps), batch N=4096, E=512, V=1024:
    emb    = probs_{t-1} @ W_d2e.T            # [N, E]
    h      = GRUCell(emb, h)                  # [N, E]
    logits = h @ W_e2d.T + b_e2d              # [N, V]
    probs  = softmax(logits)                  # [N, V]  -> output[t]

Sharding: data-parallel over N across 8 cores (512 rows each), weights
replicated, the 32-step scan stays local per core — no collectives.

On-chip layout is feature-major ([features on partitions, batch on free])
so every matmul chains without transposes; the per-core output is written
feature-major as [T, V, 512] and un-transposed on the host during the
gather.  All matmuls run as float32r (fp32 data rounded to the PE's fast
fp32 format: ~1.6e-4 relative, ~2x faster than strict fp32).  Softmax
row sums (a reduction over partitions) are computed with a ones-MATRIX
matmul whose output lands pre-broadcast on all 128 partitions (matmul
cost scales only with the free dim, so M=128 costs the same as M=1);
the reciprocal then runs on the vector engine off the PE's critical
path.  The normalization of the NEXT step's emb input is folded into
the PSUM->SBUF drain of the emb matmul (exp(logits) @ W scaled by
1/sum == softmax @ W), so the unnormalized exp tiles feed the matmul
and the fp32 output tiles get a full-precision normalize.
"""

import sys
import types

import numpy as np

import concourse.bacc as bacc
import concourse.mybir as mybir
import concourse.tile as tile

F32 = mybir.dt.float32
F32R = mybir.dt.float32r
AF = mybir.ActivationFunctionType

N_CORES = 8


def _install_ntff_hook():
    """Register the axon NTFF profiling hook if the image's antenv lacks it."""
    try:
        import antenv.axon_hooks  # noqa: F401
        return
    except ImportError:
        pass
    try:
        from trn_agent_boot.trn_boot import _ntff_profile_via_ctypes

        hook = _ntff_profile_via_ctypes("/opt/axon/libaxon_pjrt.so")
    except Exception:
        hook = None
    mod = types.ModuleType("antenv.axon_hooks")
    mod.get_axon_ntff_profile_hook = lambda: hook
    mod.set_axon_ntff_profile_hook = lambda h: None
    sys.modules["antenv.axon_hooks"] = mod


_install_ntff_hook()


def _build(T, B, E, V):
    """Build the per-core Bacc module. B = per-core batch (free dim)."""
    KE = E // 128  # E-tiles (4)
    KV = V // 128  # V-tiles (8)

    nc = bacc.Bacc(None, target_bir_lowering=False)

    xT = nc.dram_tensor("xT", [E, B], F32, kind="ExternalInput")
    wd2eT = nc.dram_tensor("wd2eT", [V, E], F32, kind="ExternalInput")
    wihT = nc.dram_tensor("wihT", [E, 3 * E], F32, kind="ExternalInput")
    whhT = nc.dram_tensor("whhT", [E, 3 * E], F32, kind="ExternalInput")
    we2dT = nc.dram_tensor("we2dT", [E, V], F32, kind="ExternalInput")
    brz = nc.dram_tensor("brz", [128, 2 * KE], F32, kind="ExternalInput")
    bihn = nc.dram_tensor("bihn", [128, KE], F32, kind="ExternalInput")
    bhhn = nc.dram_tensor("bhhn", [128, KE], F32, kind="ExternalInput")
    be2d = nc.dram_tensor("be2d", [128, KV], F32, kind="ExternalInput")
    out = nc.dram_tensor("out", [T, V, B], F32, kind="ExternalOutput")

    with tile.TileContext(nc) as tc:
        with (
            tc.tile_pool(name="w", bufs=1) as wp,
            tc.tile_pool(name="sb", bufs=1) as sb,
            tc.tile_pool(name="ps", bufs=1, space="PSUM") as pp,
        ):
            # ---- persistent weights (rounded to f32r via a copy) ----
            def load_f32r(name, dram_ap, rows, cols):
                st = sb.tile([128, cols], F32, name="stage", tag="stage", bufs=2)
                nc.sync.dma_start(st[:, :], dram_ap)
                wt = wp.tile([128, cols], F32R, name=name, tag=name)
                nc.vector.tensor_copy(wt[:], st[:])
                return wt

            w_d2e = [
                load_f32r(f"w_d2e{k}", wd2eT[k * 128 : (k + 1) * 128, :], 128, E)
                for k in range(KV)
            ]
            w_ih = [
                load_f32r(f"w_ih{k}", wihT[k * 128 : (k + 1) * 128, :], 128, 3 * E)
                for k in range(KE)
            ]
            w_hh = [
                load_f32r(f"w_hh{k}", whhT[k * 128 : (k + 1) * 128, :], 128, 3 * E)
                for k in range(KE)
            ]
            w_e2d = [
                load_f32r(f"w_e2d{k}", we2dT[k * 128 : (k + 1) * 128, :], 128, V)
                for k in range(KE)
            ]

            b_rz = wp.tile([128, 2 * KE], F32, name="b_rz", tag="b_rz")
            nc.sync.dma_start(b_rz[:], brz[:])
            b_ihn = wp.tile([128, KE], F32, name="b_ihn", tag="b_ihn")
            nc.sync.dma_start(b_ihn[:], bihn[:])
            b_hhn = wp.tile([128, KE], F32, name="b_hhn", tag="b_hhn")
            nc.sync.dma_start(b_hhn[:], bhhn[:])
            b_e2d = wp.tile([128, KV], F32, name="b_e2d", tag="b_e2d")
            nc.sync.dma_start(b_e2d[:], be2d[:])

            ones_f32 = wp.tile([128, 128], F32, name="ones_f32", tag="ones_f32")
            nc.gpsimd.memset(ones_f32[:], 1.0)
            ones_mat = wp.tile([128, 128], F32R, name="ones_mat", tag="ones_mat")
            nc.vector.tensor_copy(ones_mat[:], ones_f32[:])

            # ---- initial state h = x (feature-major, rounded to f32r) ----
            hT = []
            for m in range(KE):
                st = sb.tile([128, B], F32, name="stage", tag="stage", bufs=2)
                nc.sync.dma_start(st[:, :B], xT[m * 128 : (m + 1) * 128, :])
                ht = sb.tile([128, B], F32R, name="h", tag="h", bufs=8)
                nc.vector.tensor_copy(ht[:], st[:, :B])
                hT.append(ht)

            eT = None  # unnormalized exp(logits) of previous step (f32r)
            rbc = None  # 1/rowsum broadcast [128, B] of previous step

            for t in range(T):
                # ---- emb = softmax_{t-1} @ W_d2e.T  (feature-major [E, B]);
                # normalization folded into the PSUM drain ----
                embT = None
                if t > 0:
                    embT = []
                    for m in range(KE):
                        ps = pp.tile([128, B], F32, name="ps_mm", tag="mm", bufs=7)
                        for k in range(KV):
                            nc.tensor.matmul(
                                ps[:],
                                w_d2e[k][:, m * 128 : (m + 1) * 128],
                                eT[k][:],
                                start=(k == 0),
                                stop=(k == KV - 1),
                            )
                        ev = sb.tile([128, B], F32R, name="embT", tag="embT", bufs=8)
                        nc.vector.tensor_mul(ev[:], ps[:], rbc[:])
                        embT.append(ev)

                # ---- gates r, z: sigmoid(gx + gh + b_ih + b_hh) ----
                rz = []
                for g in range(2):
                    gt_list = []
                    for m in range(KE):
                        col = g * E + m * 128
                        ps = pp.tile([128, B], F32, name="ps_mm", tag="mm", bufs=7)
                        first = True
                        if t > 0:
                            for k in range(KE):
                                nc.tensor.matmul(
                                    ps[:],
                                    w_ih[k][:, col : col + 128],
                                    embT[k][:],
                                    start=first,
                                    stop=False,
                                )
                                first = False
                        for k in range(KE):
                            nc.tensor.matmul(
                                ps[:],
                                w_hh[k][:, col : col + 128],
                                hT[k][:],
                                start=first,
                                stop=(k == KE - 1),
                            )
                            first = False
                        gt = sb.tile(
                            [128, B], F32, name=f"gate{g}", tag=f"gate{g}", bufs=4
                        )
                        j = g * KE + m
                        nc.scalar.activation(
                            gt[:], ps[:], AF.Sigmoid, bias=b_rz[:, j : j + 1]
                        )
                        gt_list.append(gt)
                    rz.append(gt_list)
                r_g, z_g = rz

                # ---- n gate: tanh(xn + b_ihn + r * (hn + b_hhn)) ----
                xnb = None
                if t > 0:
                    xnb = []
                    for m in range(KE):
                        col = 2 * E + m * 128
                        ps = pp.tile([128, B], F32, name="ps_mm", tag="mm", bufs=7)
                        for k in range(KE):
                            nc.tensor.matmul(
                                ps[:],
                                w_ih[k][:, col : col + 128],
                                embT[k][:],
                                start=(k == 0),
                                stop=(k == KE - 1),
                            )
                        xv = sb.tile([128, B], F32, name="xnb", tag="xnb", bufs=4)
                        nc.scalar.activation(
                            xv[:], ps[:], AF.Identity, bias=b_ihn[:, m : m + 1]
                        )
                        xnb.append(xv)

                n_g = []
                for m in range(KE):
                    col = 2 * E + m * 128
                    ps = pp.tile([128, B], F32, name="ps_mm", tag="mm", bufs=7)
                    for k in range(KE):
                        nc.tensor.matmul(
                            ps[:],
                            w_hh[k][:, col : col + 128],
                            hT[k][:],
                            start=(k == 0),
                            stop=(k == KE - 1),
                        )
                    hv = sb.tile([128, B], F32, name="hnb", tag="hnb", bufs=4)
                    nc.scalar.activation(
                        hv[:], ps[:], AF.Identity, bias=b_hhn[:, m : m + 1]
                    )
                    n_g.append(hv)

                for m in range(KE):
                    # t1 = r * (hn + b_hhn), in place into n_g
                    nc.vector.tensor_mul(n_g[m][:], r_g[m][:], n_g[m][:])
                if t > 0:
                    for m in range(KE):
                        nc.vector.tensor_add(n_g[m][:], n_g[m][:], xnb[m][:])
                    for m in range(KE):
                        nc.scalar.activation(n_g[m][:], n_g[m][:], AF.Tanh)
                else:
                    # xn == 0 at t=0; fold b_ihn into the tanh bias
                    for m in range(KE):
                        nc.scalar.activation(
                            n_g[m][:], n_g[m][:], AF.Tanh, bias=b_ihn[:, m : m + 1]
                        )

                # ---- h' = n + z * (h - n) ----
                hN = []
                for m in range(KE):
                    nc.vector.tensor_sub(
                        r_g[m][:], hT[m][:].bitcast(F32), n_g[m][:]
                    )  # d = h - n (reuse r tile)
                    nc.vector.tensor_mul(z_g[m][:], z_g[m][:], r_g[m][:])  # z*d
                    ht = sb.tile([128, B], F32R, name="h", tag="h", bufs=8)
                    nc.vector.tensor_add(ht[:], n_g[m][:], z_g[m][:])
                    hN.append(ht)
                hT = hN

                # ---- logits = h' @ W_e2d.T + b_e2d; eT = exp(logits) ----
                eT = []
                for j in range(KV):
                    ps = pp.tile([128, B], F32, name="ps_mm", tag="mm", bufs=7)
                    for k in range(KE):
                        nc.tensor.matmul(
                            ps[:],
                            w_e2d[k][:, j * 128 : (j + 1) * 128],
                            hT[k][:],
                            start=(k == 0),
                            stop=(k == KE - 1),
                        )
                    ev = sb.tile([128, B], F32R, name="eT", tag="eT", bufs=12)
                    nc.scalar.activation(
                        ev[:], ps[:], AF.Exp, bias=b_e2d[:, j : j + 1]
                    )
                    eT.append(ev)

                # ---- softmax row sums over V (partition reduction), broadcast
                # to all 128 partitions by using a ones MATRIX as stationary
                # (same cost as a ones vector: matmul time scales with the
                # free dim only).  The PE never waits on the reciprocal. ----
                ps_s = pp.tile([128, B], F32, name="ps_s", tag="srow", bufs=1)
                for j in range(KV):
                    nc.tensor.matmul(
                        ps_s[:],
                        ones_mat[:],
                        eT[j][:],
                        start=(j == 0),
                        stop=(j == KV - 1),
                    )
                rbc = sb.tile([128, B], F32, name="rbc", tag="rbc", bufs=2)
                nc.vector.reciprocal(rbc[:], ps_s[:])

                # ---- probs = exp(logits) / rowsum -> output[t] (fp32) ----
                for j in range(KV):
                    po = sb.tile([128, B], F32, name="pout", tag="pout", bufs=6)
                    nc.vector.tensor_mul(po[:], eT[j][:].bitcast(F32), rbc[:])
                    nc.sync.dma_start(out[t, j * 128 : (j + 1) * 128, :], po[:])

    nc.compile()
    return nc


def _prep_inputs(x, W_d2e, W_ih, W_hh, b_ih, b_hh, W_e2d, b_e2d):
    E = x.shape[1]
    V = np.asarray(W_e2d).shape[0]
    KE = E // 128
    KV = V // 128

    def c(a):
        return np.ascontiguousarray(np.asarray(a), dtype=np.float32)

    shared = {
        "wd2eT": c(np.asarray(W_d2e).T),  # [V, E]
        "wihT": c(np.asarray(W_ih).T),  # [E, 3E]
        "whhT": c(np.asarray(W_hh).T),
        "we2dT": c(np.asarray(W_e2d).T),  # [E, V]
        "brz": c((np.asarray(b_ih) + np.asarray(b_hh))[: 2 * E].reshape(2 * KE, 128).T),
        "bihn": c(np.asarray(b_ih)[2 * E :].reshape(KE, 128).T),
        "bhhn": c(np.asarray(b_hh)[2 * E :].reshape(KE, 128).T),
        "be2d": c(np.asarray(b_e2d).reshape(KV, 128).T),
    }
    N = x.shape[0]
    B = N // N_CORES
    in_maps = []
    for core in range(N_CORES):
        m = dict(shared)
        m["xT"] = c(np.asarray(x)[core * B : (core + 1) * B, :].T)  # [E, B]
        in_maps.append(m)
    return in_maps, B


def _run(inputs, trace=False):
    from concourse.bass_utils import run_bass_kernel_spmd

    x = np.asarray(inputs["x"], dtype=np.float32)
    T = int(inputs["max_len"])
    N, E = x.shape
    V = np.asarray(inputs["W_e2d"]).shape[0]
    assert N % N_CORES == 0 and E % 128 == 0 and V % 128 == 0

    in_maps, B = _prep_inputs(
        x,
        inputs["W_d2e"],
        inputs["W_ih"],
        inputs["W_hh"],
        inputs["b_ih"],
        inputs["b_hh"],
        inputs["W_e2d"],
        inputs["b_e2d"],
    )
    nc = _build(T, B, E, V)
    res = run_bass_kernel_spmd(
        nc, in_maps, core_ids=list(range(N_CORES)), trace=trace
    )

    full = np.empty((T, N, V), dtype=np.float32)
    for core in range(N_CORES):
        o = res.results[core]["out"]  # [T, V, B]
        full[:, core * B : (core + 1) * B, :] = np.transpose(o, (0, 2, 1))
    return full, res


def kernel(**inputs):
    full, _ = _run(inputs, trace=False)
    return full


def run_traced(**inputs):
    return _run(inputs, trace=True)


# revision 9
# speedup vs baseline: 1.4197x; 1.1890x over previous
"""Trainium2 Bass kernel for a differentiable GRU decoder.

Per step t (max_len=32 steps), batch N=4096, E=512, V=1024:
    emb    = probs_{t-1} @ W_d2e.T            # [N, E]
    h      = GRUCell(emb, h)                  # [N, E]
    logits = h @ W_e2d.T + b_e2d              # [N, V]
    probs  = softmax(logits)                  # [N, V]  -> output[t]

Sharding: data-parallel over N across 8 cores (512 rows each), weights
replicated, the 32-step scan stays local per core — no collectives.

Design notes:
- Feature-major on-chip layout ([features on partitions, batch on free])
  lets every matmul chain without transposes; the per-core output is
  written feature-major as [T, V, 512] and un-transposed on the host
  during the gather.
- Matmul operands stream as bf16 by default (DEC_MM_DT=f32r switches to
  the fp32r path: ~2x slower, ~1.6e-4 output error vs ~2e-3 for bf16).
  PSUM accumulation is fp32 either way.  The GRU state keeps an fp32
  master copy for the elementwise update; a rounded copy feeds the PE.
- Softmax row sums (a reduction over partitions) are computed with a
  ones-MATRIX matmul whose output lands pre-broadcast on all 128
  partitions (matmul cost scales only with the free dim, so M=128 costs
  the same as M=1); the reciprocal runs on the vector engine off the
  PE's critical path.  exp(logits) stays unnormalized: the 1/sum scale
  folds into the PSUM drain of the next step's emb matmul (per-batch
  scaling commutes with the contraction), and the fp32 output tiles get
  a full-precision normalize whose emission is deferred behind the next
  step's matmuls so it never blocks the PE.
- In each gate's PSUM accumulation the recurrent (W_hh @ h) half is
  emitted before the (W_ih @ emb) half, giving the scheduler
  emb-independent matmuls to run while the softmax reciprocal chain
  resolves.
"""

import os
import sys
import types

import numpy as np

import concourse.bacc as bacc
import concourse.mybir as mybir
import concourse.tile as tile

F32 = mybir.dt.float32
F32R = mybir.dt.float32r
BF16 = mybir.dt.bfloat16
AF = mybir.ActivationFunctionType

N_CORES = 8
MM_DT = F32R if os.environ.get("DEC_MM_DT", "bf16") == "f32r" else BF16


def _install_ntff_hook():
    """Register the axon NTFF profiling hook if the image's antenv lacks it."""
    try:
        import antenv.axon_hooks  # noqa: F401
        return
    except ImportError:
        pass
    try:
        from trn_agent_boot.trn_boot import _ntff_profile_via_ctypes

        hook = _ntff_profile_via_ctypes("/opt/axon/libaxon_pjrt.so")
    except Exception:
        hook = None
    mod = types.ModuleType("antenv.axon_hooks")
    mod.get_axon_ntff_profile_hook = lambda: hook
    mod.set_axon_ntff_profile_hook = lambda h: None
    sys.modules["antenv.axon_hooks"] = mod


_install_ntff_hook()


def _build(T, B, E, V):
    """Build the per-core Bacc module. B = per-core batch (free dim)."""
    KE = E // 128  # E-tiles (4)
    KV = V // 128  # V-tiles (8)

    nc = bacc.Bacc(None, target_bir_lowering=False)

    wdt = F32 if MM_DT == F32R else BF16  # dram dtype for weights
    xT = nc.dram_tensor("xT", [E, B], F32, kind="ExternalInput")
    wd2eT = nc.dram_tensor("wd2eT", [V, E], wdt, kind="ExternalInput")
    wihT = nc.dram_tensor("wihT", [E, 3 * E], wdt, kind="ExternalInput")
    whhT = nc.dram_tensor("whhT", [E, 3 * E], wdt, kind="ExternalInput")
    we2dT = nc.dram_tensor("we2dT", [E, V], wdt, kind="ExternalInput")
    brz = nc.dram_tensor("brz", [128, 2 * KE], F32, kind="ExternalInput")
    bihn = nc.dram_tensor("bihn", [128, KE], F32, kind="ExternalInput")
    bhhn = nc.dram_tensor("bhhn", [128, KE], F32, kind="ExternalInput")
    be2d = nc.dram_tensor("be2d", [128, KV], F32, kind="ExternalInput")
    out = nc.dram_tensor("out", [T, V, B], F32, kind="ExternalOutput")

    with tile.TileContext(nc) as tc:
        with (
            tc.tile_pool(name="w", bufs=1) as wp,
            tc.tile_pool(name="sb", bufs=1) as sb,
            tc.tile_pool(name="ps", bufs=1, space="PSUM") as pp,
        ):
            # ---- persistent weights, in first-use order (w_hh feeds t=0) ----
            def load_w(name, dram_ap, cols):
                if MM_DT == BF16:
                    wt = wp.tile([128, cols], BF16, name=name, tag=name)
                    nc.sync.dma_start(wt[:], dram_ap)
                else:
                    st = sb.tile([128, cols], F32, name="stage", tag="stage", bufs=2)
                    nc.sync.dma_start(st[:], dram_ap)
                    wt = wp.tile([128, cols], F32R, name=name, tag=name)
                    nc.vector.tensor_copy(wt[:], st[:])
                return wt

            w_hh = [
                load_w(f"w_hh{k}", whhT[k * 128 : (k + 1) * 128, :], 3 * E)
                for k in range(KE)
            ]
            w_e2d = [
                load_w(f"w_e2d{k}", we2dT[k * 128 : (k + 1) * 128, :], V)
                for k in range(KE)
            ]
            w_d2e = [
                load_w(f"w_d2e{k}", wd2eT[k * 128 : (k + 1) * 128, :], E)
                for k in range(KV)
            ]
            w_ih = [
                load_w(f"w_ih{k}", wihT[k * 128 : (k + 1) * 128, :], 3 * E)
                for k in range(KE)
            ]

            b_rz = wp.tile([128, 2 * KE], F32, name="b_rz", tag="b_rz")
            nc.sync.dma_start(b_rz[:], brz[:])
            b_ihn = wp.tile([128, KE], F32, name="b_ihn", tag="b_ihn")
            nc.sync.dma_start(b_ihn[:], bihn[:])
            b_hhn = wp.tile([128, KE], F32, name="b_hhn", tag="b_hhn")
            nc.sync.dma_start(b_hhn[:], bhhn[:])
            b_e2d = wp.tile([128, KV], F32, name="b_e2d", tag="b_e2d")
            nc.sync.dma_start(b_e2d[:], be2d[:])

            ones_f32 = wp.tile([128, 128], F32, name="ones_f32", tag="ones_f32")
            nc.gpsimd.memset(ones_f32[:], 1.0)
            ones_mat = wp.tile([128, 128], MM_DT, name="ones_mat", tag="ones_mat")
            nc.vector.tensor_copy(ones_mat[:], ones_f32[:])

            # ---- initial state h = x: fp32 master + MM_DT copy for the PE ----
            hT = []  # fp32 master
            hM = []  # MM_DT matmul copy
            for m in range(KE):
                hf = sb.tile([128, B], F32, name="h", tag="h", bufs=8)
                nc.sync.dma_start(hf[:], xT[m * 128 : (m + 1) * 128, :])
                hT.append(hf)
                hm = sb.tile([128, B], MM_DT, name="hmm", tag="hmm", bufs=8)
                nc.scalar.copy(hm[:], hf[:])
                hM.append(hm)

            eT = None  # unnormalized exp(logits) of previous step (MM_DT)
            rbc = None  # 1/rowsum broadcast [128, B] fp32
            deferred_out = None  # (t, eT, rbc) waiting for output normalize

            def emit_pout(item):
                t_prev, eT_prev, rbc_prev = item
                for j in range(KV):
                    po = sb.tile([128, B], F32, name="pout", tag="pout", bufs=6)
                    nc.vector.tensor_mul(po[:], eT_prev[j][:], rbc_prev[:])
                    nc.sync.dma_start(
                        out[t_prev, j * 128 : (j + 1) * 128, :], po[:]
                    )

            for t in range(T):
                # ---- emb = softmax_{t-1} @ W_d2e.T (feature-major [E, B]);
                # normalization folded into the PSUM drain ----
                embT = None
                if t > 0:
                    embT = []
                    for m in range(KE):
                        ps = pp.tile([128, B], F32, name="ps_mm", tag="mm", bufs=7)
                        for k in range(KV):
                            nc.tensor.matmul(
                                ps[:],
                                w_d2e[k][:, m * 128 : (m + 1) * 128],
                                eT[k][:],
                                start=(k == 0),
                                stop=(k == KV - 1),
                            )
                        ev = sb.tile([128, B], MM_DT, name="embT", tag="embT", bufs=8)
                        nc.vector.tensor_mul(ev[:], ps[:], rbc[:])
                        embT.append(ev)

                # ---- gates r, z: sigmoid(gh + gx + biases); gh emitted first
                # so the PE has emb-independent work during the softmax tail ----
                rz = []
                for g in range(2):
                    gt_list = []
                    for m in range(KE):
                        col = g * E + m * 128
                        ps = pp.tile([128, B], F32, name="ps_mm", tag="mm", bufs=7)
                        for k in range(KE):
                            nc.tensor.matmul(
                                ps[:],
                                w_hh[k][:, col : col + 128],
                                hM[k][:],
                                start=(k == 0),
                                stop=(t == 0 and k == KE - 1),
                            )
                        if t > 0:
                            for k in range(KE):
                                nc.tensor.matmul(
                                    ps[:],
                                    w_ih[k][:, col : col + 128],
                                    embT[k][:],
                                    start=False,
                                    stop=(k == KE - 1),
                                )
                        gt = sb.tile(
                            [128, B], F32, name=f"gate{g}", tag=f"gate{g}", bufs=4
                        )
                        j = g * KE + m
                        nc.scalar.activation(
                            gt[:], ps[:], AF.Sigmoid, bias=b_rz[:, j : j + 1]
                        )
                        gt_list.append(gt)
                    rz.append(gt_list)
                r_g, z_g = rz

                # ---- n gate: tanh(xn + b_ihn + r * (hn + b_hhn)) ----
                n_g = []
                for m in range(KE):
                    col = 2 * E + m * 128
                    ps = pp.tile([128, B], F32, name="ps_mm", tag="mm", bufs=7)
                    for k in range(KE):
                        nc.tensor.matmul(
                            ps[:],
                            w_hh[k][:, col : col + 128],
                            hM[k][:],
                            start=(k == 0),
                            stop=(k == KE - 1),
                        )
                    hv = sb.tile([128, B], F32, name="hnb", tag="hnb", bufs=4)
                    nc.scalar.activation(
                        hv[:], ps[:], AF.Identity, bias=b_hhn[:, m : m + 1]
                    )
                    n_g.append(hv)

                xnb = None
                if t > 0:
                    xnb = []
                    for m in range(KE):
                        col = 2 * E + m * 128
                        ps = pp.tile([128, B], F32, name="ps_mm", tag="mm", bufs=7)
                        for k in range(KE):
                            nc.tensor.matmul(
                                ps[:],
                                w_ih[k][:, col : col + 128],
                                embT[k][:],
                                start=(k == 0),
                                stop=(k == KE - 1),
                            )
                        xv = sb.tile([128, B], F32, name="xnb", tag="xnb", bufs=4)
                        nc.scalar.activation(
                            xv[:], ps[:], AF.Identity, bias=b_ihn[:, m : m + 1]
                        )
                        xnb.append(xv)

                # output normalize + store of the PREVIOUS step: emitted here
                # so it sits behind this step's emb drain in the DVE queue
                if deferred_out is not None:
                    emit_pout(deferred_out)
                    deferred_out = None

                for m in range(KE):
                    # t1 = r * (hn + b_hhn), in place into n_g
                    nc.vector.tensor_mul(n_g[m][:], r_g[m][:], n_g[m][:])
                if t > 0:
                    for m in range(KE):
                        nc.vector.tensor_add(n_g[m][:], n_g[m][:], xnb[m][:])
                    for m in range(KE):
                        nc.scalar.activation(n_g[m][:], n_g[m][:], AF.Tanh)
                else:
                    # xn == 0 at t=0; fold b_ihn into the tanh bias
                    for m in range(KE):
                        nc.scalar.activation(
                            n_g[m][:], n_g[m][:], AF.Tanh, bias=b_ihn[:, m : m + 1]
                        )

                # ---- h' = n + z * (h - n); fp32 master + MM_DT copy ----
                hN = []
                hNM = []
                for m in range(KE):
                    nc.vector.tensor_sub(r_g[m][:], hT[m][:], n_g[m][:])  # h - n
                    nc.vector.tensor_mul(z_g[m][:], z_g[m][:], r_g[m][:])
                    hf = sb.tile([128, B], F32, name="h", tag="h", bufs=8)
                    nc.vector.tensor_add(hf[:], n_g[m][:], z_g[m][:])
                    hN.append(hf)
                    hm = sb.tile([128, B], MM_DT, name="hmm", tag="hmm", bufs=8)
                    nc.scalar.copy(hm[:], hf[:])
                    hNM.append(hm)
                hT = hN
                hM = hNM

                # ---- logits = h' @ W_e2d.T + b_e2d; eT = exp(logits) ----
                eT = []
                for j in range(KV):
                    ps = pp.tile([128, B], F32, name="ps_mm", tag="mm", bufs=7)
                    for k in range(KE):
                        nc.tensor.matmul(
                            ps[:],
                            w_e2d[k][:, j * 128 : (j + 1) * 128],
                            hM[k][:],
                            start=(k == 0),
                            stop=(k == KE - 1),
                        )
                    ev = sb.tile([128, B], MM_DT, name="eT", tag="eT", bufs=12)
                    nc.scalar.activation(
                        ev[:], ps[:], AF.Exp, bias=b_e2d[:, j : j + 1]
                    )
                    eT.append(ev)

                # ---- softmax row sums over V, pre-broadcast via ones matrix ----
                ps_s = pp.tile([128, B], F32, name="ps_s", tag="srow", bufs=1)
                for j in range(KV):
                    nc.tensor.matmul(
                        ps_s[:],
                        ones_mat[:],
                        eT[j][:],
                        start=(j == 0),
                        stop=(j == KV - 1),
                    )
                rbc = sb.tile([128, B], F32, name="rbc", tag="rbc", bufs=2)
                nc.vector.reciprocal(rbc[:], ps_s[:])

                deferred_out = (t, eT, rbc)

            emit_pout(deferred_out)

    nc.compile()
    return nc


def _prep_inputs(x, W_d2e, W_ih, W_hh, b_ih, b_hh, W_e2d, b_e2d):
    E = x.shape[1]
    V = np.asarray(W_e2d).shape[0]
    KE = E // 128
    KV = V // 128

    if MM_DT == BF16:
        import ml_dtypes

        wnp = ml_dtypes.bfloat16
    else:
        wnp = np.float32

    def c(a, dt=np.float32):
        return np.ascontiguousarray(np.asarray(a, dtype=np.float32).astype(dt))

    shared = {
        "wd2eT": c(np.asarray(W_d2e).T, wnp),  # [V, E]
        "wihT": c(np.asarray(W_ih).T, wnp),  # [E, 3E]
        "whhT": c(np.asarray(W_hh).T, wnp),
        "we2dT": c(np.asarray(W_e2d).T, wnp),  # [E, V]
        "brz": c((np.asarray(b_ih) + np.asarray(b_hh))[: 2 * E].reshape(2 * KE, 128).T),
        "bihn": c(np.asarray(b_ih)[2 * E :].reshape(KE, 128).T),
        "bhhn": c(np.asarray(b_hh)[2 * E :].reshape(KE, 128).T),
        "be2d": c(np.asarray(b_e2d).reshape(KV, 128).T),
    }
    N = x.shape[0]
    B = N // N_CORES
    in_maps = []
    for core in range(N_CORES):
        m = dict(shared)
        m["xT"] = c(np.asarray(x)[core * B : (core + 1) * B, :].T)  # [E, B]
        in_maps.append(m)
    return in_maps, B


def _run(inputs, trace=False):
    from concourse.bass_utils import run_bass_kernel_spmd

    x = np.asarray(inputs["x"], dtype=np.float32)
    T = int(inputs["max_len"])
    N, E = x.shape
    V = np.asarray(inputs["W_e2d"]).shape[0]
    assert N % N_CORES == 0 and E % 128 == 0 and V % 128 == 0

    in_maps, B = _prep_inputs(
        x,
        inputs["W_d2e"],
        inputs["W_ih"],
        inputs["W_hh"],
        inputs["b_ih"],
        inputs["b_hh"],
        inputs["W_e2d"],
        inputs["b_e2d"],
    )
    nc = _build(T, B, E, V)
    res = run_bass_kernel_spmd(
        nc, in_maps, core_ids=list(range(N_CORES)), trace=trace
    )

    full = np.empty((T, N, V), dtype=np.float32)
    for core in range(N_CORES):
        o = res.results[core]["out"]  # [T, V, B]
        full[:, core * B : (core + 1) * B, :] = np.transpose(o, (0, 2, 1))
    return full, res


def kernel(**inputs):
    full, _ = _run(inputs, trace=False)
    return full


def run_traced(**inputs):
    return _run(inputs, trace=True)
